# revision 22
# baseline (speedup 1.0000x reference)
"""Multi-head attention (B=4, N=2048, C=768, H=12) on 8 trn2 NeuronCores.

Sharding: core c handles batch b = c//2 and query rows [ (c%2)*1024, +1024 ).
Each core computes K/V for its full batch (duplicated across the pair),
attention for all 12 heads over its 1024 queries, and the output projection
for its rows. Output gather is pure concatenation (no cross-core reduce).

v3 layout (per core):
  xta/xtb = x_b.T split  [768, 1024]x2  (query-half columns first for fast Q)
  QT  = Wq @ xq.T        [768, 1024]    head h rows h*64..h*64+63
  KT  = Wk @ x.T         [768, 2048]
  V   = x @ Wvi.T        [2048, 780]    Wvi host-interleaved so head h lands
                                        at cols h*65..h*65+63; col h*65+64 is
                                        memset to 1.0 (ones column -> the PV
                                        matmul also emits softmax denominators)
  attention per (q-chunk of 512, head-pair): 16 k-tiles; each iteration's
  two score matmuls (row-group-packed via tile_position) write one
  [128,1024] 2-bank PSUM pair tile, double-buffered so ST(i+1) overlaps
  exp(i). exp runs pair-wide on ScalarE (exact, 1/8 scale folded) except
  head1 of iters {5,7,9,11,13,14}, which runs on the DVE via a mean-centered
  sum-of-two-Schraudolph bit-trick exp (int16 convert, bitcast to bf16) so
  ScalarE stays strictly faster than the PE per iteration (keeps HAM warm).
  The ones column of V makes the PV matmul emit softmax denominators, which
  are inverted with reciprocal_approx_fast and broadcast on GpSimd; the
  normalize multiply runs on the DVE straight out of PSUM.
  Y = OT^T Wp^T + bp     [1024, 768], bias added during PSUM evacuation
  (GpSimd-broadcast bias tile), first 4 q-tiles emitted between the two
  q-chunk passes so projection overlaps attention.
"""

import os
import sys

import numpy as np

sys.path.insert(0, "/opt/trn_rl_repo")

import concourse.bass as bass
from concourse import bacc
import concourse.mybir as mybir
from concourse.tile import TileContext
from concourse.bass_utils import run_bass_kernel_spmd
from concourse.dma_utils import dma_copy

P = 128
C = 768
NK = 2048
NQ = 1024
H = 12
DH = 64
CT = C // P          # 6 c-tiles (contraction tiles for the linears)
KT = NK // P         # 16 key tiles
QCH = 512            # q-chunk (1 psum bank of fp32)
NQC = NQ // QCH      # 2 q chunks
VW = H * (DH + 1)    # 780: interleaved V width
SCALE = DH ** -0.5
F32 = mybir.dt.float32
BF16 = mybir.dt.bfloat16
I16 = mybir.dt.int16

# sum-of-two-Schraudolph exp on the DVE: pt = bf16(u1) + bf16(u2) where
# u{1,2} = int16(A*s + B{1,2}); constants centered so mean(pt/exp(s*SCALE))=1
EXP_A = SCALE * np.log2(np.e) * 128.0
EXP_B1 = 16256.0 - 170.12143
DVE_EXP_I = (5, 7, 9, 11, 13, 14)   # head1-exp-on-DVE iters: kept clear of
# the first iters (DVE is still draining the previous pair's normalization)

LAST_RESULT = None
_PROG = None


def _build_program() -> bass.Bass:
    nc = bacc.Bacc(None, target_bir_lowering=False)

    xta = nc.dram_tensor("xta", [C, NQ], BF16, kind="ExternalInput")
    xtb = nc.dram_tensor("xtb", [C, NK - NQ], BF16, kind="ExternalInput")
    wqt = nc.dram_tensor("wqt", [C, C], BF16, kind="ExternalInput")
    wkt = nc.dram_tensor("wkt", [C, C], BF16, kind="ExternalInput")
    wvi = nc.dram_tensor("wvi", [C, VW], BF16, kind="ExternalInput")
    wpt = nc.dram_tensor("wpt", [C, C], BF16, kind="ExternalInput")
    bp = nc.dram_tensor("bp", [1, C], F32, kind="ExternalInput")
    y = nc.dram_tensor("y", [NQ, C], F32, kind="ExternalOutput")

    with TileContext(nc) as tc:
        with (
            tc.tile_pool(name="persist", bufs=1) as persist,
            tc.tile_pool(name="pt", bufs=6) as ptp,
            tc.tile_pool(name="u16", bufs=4) as upool,
            tc.tile_pool(name="small", bufs=2) as small,
            tc.tile_pool(name="ysb", bufs=2) as ysb,
            tc.tile_pool(name="psa", bufs=2, space="PSUM") as psa,
            tc.tile_pool(name="psb", bufs=4, space="PSUM") as psb,
        ):
            # preload the exp table set while DMAs run
            warm = persist.tile([1, 16], BF16, tag="warm")
            nc.gpsimd.memset(warm[:, :], 0.0)
            nc.scalar.activation(
                warm[:, :], warm[:, :], mybir.ActivationFunctionType.Exp
            )

            # ---- load inputs, casting to bf16 in the DMA; q-half of x and
            # Wq first so Q's matmuls start early ----
            def load_cast(dram, rows, cols, tag):
                tiles = []
                for i in range(rows // P):
                    t = persist.tile([P, cols], BF16, tag=f"{tag}{i}")
                    dma_copy(nc.gpsimd, t[:, :], dram[i * P:(i + 1) * P, :])
                    tiles.append(t)
                return tiles

            # interleave xa/wq tile loads so Q's first accumulation step
            # can start after the first (xa, wq) tile pair lands
            xa, wqb = [], []
            for i in range(CT):
                t = persist.tile([P, NQ], BF16, tag=f"xa{i}", name=f"xa{i}")
                dma_copy(nc.gpsimd, t[:, :], xta[i * P:(i + 1) * P, :])
                xa.append(t)
                t = persist.tile([P, C], BF16, tag=f"wqb{i}", name=f"wqb{i}")
                dma_copy(nc.gpsimd, t[:, :], wqt[i * P:(i + 1) * P, :])
                wqb.append(t)
            xb = load_cast(xtb, C, NK - NQ, "xb")  # remaining columns
            wkb = load_cast(wkt, C, C, "wkb")
            wvb = load_cast(wvi, C, VW, "wvb")
            wpb = load_cast(wpt, C, C, "wpb")
            bpb = persist.tile([1, C], BF16, tag="bpb")
            dma_copy(nc.gpsimd, bpb[:, :], bp[:, :])

            bpf = persist.tile([1, C], F32, tag="bpf")
            dma_copy(nc.gpsimd, bpf[:, :], bp[:, :])
            bpbB = persist.tile([P, C], F32, tag="bpbB")
            nc.gpsimd.partition_broadcast(bpbB[:, :], bpf[0:1, :])

            # ---- Q: QT[c, q] = Wq @ xq.T : lhsT=wq tile, rhs=xa ----
            qtb = [persist.tile([P, NQ], BF16, tag=f"qt{i}", name=f"qt{i}") for i in range(CT)]
            for p in range(CT // 2):
                pst = [psa.tile([P, 2 * QCH], F32, tag="a", name="ps")
                       for _ in range(2)]
                for k in range(CT):   # k outer: start on the first DMA pair
                    for t in range(2):
                        i = 2 * p + t
                        for j in range(NQC):
                            nc.tensor.matmul(
                                pst[t][:, j * QCH:(j + 1) * QCH],
                                lhsT=wqb[k][:, i * P:(i + 1) * P],
                                rhs=xa[k][:, j * QCH:(j + 1) * QCH],
                                start=(k == 0),
                                stop=(k == CT - 1),
                            )
                for t in range(2):
                    nc.any.tensor_copy(qtb[2 * p + t][:, :], pst[t][:, :])

            # ---- K: KT[c, keys]. Columns are in PERMUTED key order: the
            # 1024 xa keys first, then the 1024 xb keys. V rows use the same
            # permuted order, so attention is unchanged (softmax is
            # permutation-invariant over keys). ----
            ktb = [persist.tile([P, NK], BF16, tag=f"kt{i}", name=f"kt{i}") for i in range(CT)]
            for i in range(CT):
                for wave in range(2):
                    pss = psa.tile([P, 2 * QCH], F32, tag="a", name="pss")
                    for k in range(CT):
                        for j in range(2):
                            kc = wave * 2 + j
                            srct = xa if kc < 2 else xb
                            off = (kc % 2) * QCH
                            nc.tensor.matmul(
                                pss[:, j * QCH:(j + 1) * QCH],
                                lhsT=wkb[k][:, i * P:(i + 1) * P],
                                rhs=srct[k][:, off:off + QCH],
                                start=(k == 0),
                                stop=(k == CT - 1),
                            )
                    nc.any.tensor_copy(
                        ktb[i][:, wave * 2 * QCH:(wave + 1) * 2 * QCH], pss[:, :]
                    )

            # ---- V: V[key, 780] = x.T-tile^T @ Wvi, keys in the same
            # permuted order as KT ([xa cols then xb cols]) ----
            vtb = [persist.tile([P, VW], BF16, tag=f"v{i}", name=f"v{i}") for i in range(KT)]
            for i in range(KT):
                srct = xa if i < 8 else xb
                soff = (i % 8) * P
                pss = psa.tile([P, 2 * QCH], F32, tag="a", name="pss")
                for k in range(CT):
                    for (c0, csz) in ((0, QCH), (QCH, VW - QCH)):
                        nc.tensor.matmul(
                            pss[:, c0:c0 + csz],
                            lhsT=srct[k][:, soff:soff + P],
                            rhs=wvb[k][:, c0:c0 + csz],
                            start=(k == 0),
                            stop=(k == CT - 1),
                        )
                nc.any.tensor_copy(vtb[i][:, 0:VW], pss[:, 0:VW])
                # ones columns at h*65+64
                onescols = vtb[i][:, :].rearrange(
                    "p (h e) -> p h e", e=DH + 1
                )[:, :, DH:DH + 1]
                nc.gpsimd.memset(onescols, 1.0)

            # ---- attention: qc outer so the first projection half can
            # overlap the second attention half ----
            otb = [persist.tile([P, NQ], BF16, tag=f"ot{hp}", name=f"ot{hp}") for hp in range(CT)]

            def proj_half(q0, q1):
                for qi in range(q0, q1):
                    yt = ysb.tile([P, C], F32, tag="y", name="yt")
                    pj = psa.tile([P, 2 * QCH], F32, tag="a", name="pj")
                    for k in range(CT):
                        for (c0, csz) in ((0, QCH), (QCH, C - QCH)):
                            nc.tensor.matmul(
                                pj[:, c0:c0 + csz],
                                lhsT=otb[k][:, qi * P:(qi + 1) * P],
                                rhs=wpb[k][:, c0:c0 + csz],
                                start=(k == 0), stop=(k == CT - 1),
                            )
                    for (c0, csz) in ((0, QCH), (QCH, C - QCH)):
                        nc.vector.tensor_add(
                            yt[:, c0:c0 + csz], pj[:, c0:c0 + csz],
                            bpbB[:, c0:c0 + csz],
                        )
                    nc.sync.dma_start(out=y[qi * P:(qi + 1) * P, :], in_=yt[:, :])

            for qc in range(NQC):
                qoff = qc * QCH
                for hp in range(CT):
                    h0, h1 = 2 * hp, 2 * hp + 1
                    ot0 = psb.tile([DH + 1, QCH], F32, tag="b", name="ot0")
                    ot1 = psb.tile([DH + 1, QCH], F32, tag="b", name="ot1")

                    def emit_av(i, pt):
                        v0 = vtb[i][:, h0 * (DH + 1):(h0 + 1) * (DH + 1)]
                        v1 = vtb[i][:, h1 * (DH + 1):(h1 + 1) * (DH + 1)]
                        nc.tensor.matmul(
                            ot0[:, :], lhsT=v0, rhs=pt[:, 0:QCH],
                            start=(i == 0), stop=(i == KT - 1),
                        )
                        nc.tensor.matmul(
                            ot1[:, :], lhsT=v1, rhs=pt[:, QCH:2 * QCH],
                            start=(i == 0), stop=(i == KT - 1),
                        )

                    pending = None
                    for i in range(KT):
                        st = psa.tile([P, 2 * QCH], F32, tag="a", name="st")
                        w0 = ktb[hp][0:DH, i * P:(i + 1) * P]
                        w1 = ktb[hp][DH:P, i * P:(i + 1) * P]
                        nc.tensor.matmul(
                            st[:, 0:QCH],
                            lhsT=w0,
                            rhs=qtb[hp][0:DH, qoff:qoff + QCH],
                            start=True, stop=True,
                            tile_position=(0, 0),
                        )
                        nc.tensor.matmul(
                            st[:, QCH:2 * QCH],
                            lhsT=w1,
                            rhs=qtb[hp][DH:P, qoff:qoff + QCH],
                            start=True, stop=True,
                            tile_position=(64, 0),
                        )
                        if pending is not None:
                            emit_av(*pending)
                        pt = ptp.tile([P, 2 * QCH], BF16, tag="pt", name="pt")
                        if i in DVE_EXP_I:
                            # head0 on ScalarE, head1 on the DVE (no spike)
                            nc.scalar.activation(
                                pt[:, 0:QCH], st[:, 0:QCH],
                                mybir.ActivationFunctionType.Exp, scale=SCALE,
                            )
                            u1 = upool.tile([P, QCH], I16, tag="u", name="u1")
                            u2 = upool.tile([P, QCH], I16, tag="u", name="u2")
                            nc.vector.tensor_scalar(
                                u1[:, :], st[:, QCH:2 * QCH], EXP_A, EXP_B1,
                                mybir.AluOpType.mult, mybir.AluOpType.add,
                            )
                            nc.vector.tensor_scalar_add(u2[:, :], u1[:, :], 64)
                            nc.vector.tensor_add(
                                pt[:, QCH:2 * QCH],
                                u1[:, :].bitcast(BF16),
                                u2[:, :].bitcast(BF16),
                            )
                        else:
                            nc.scalar.activation(
                                pt[:, :], st[:, :],
                                mybir.ActivationFunctionType.Exp, scale=SCALE,
                            )
                        pending = (i, pt)
                    emit_av(*pending)

                    last_seg = (qc == NQC - 1 and hp == CT - 1)
                    for idx, (ot, hh) in enumerate(((ot0, 0), (ot1, 1))):
                        den = small.tile([1, QCH], F32, tag=f"den{idx}", name="den")
                        if last_seg:
                            nc.scalar.copy(den[0:1, :], ot[DH:DH + 1, :])
                        else:
                            nc.vector.tensor_copy(den[0:1, :], ot[DH:DH + 1, :])
                        rec = small.tile([1, QCH], F32, tag=f"rec{idx}", name="rec")
                        nc.vector.reciprocal_approx_fast(
                            out=rec[0:1, :], in_=den[0:1, :]
                        )
                        recB = small.tile([DH, QCH], F32, tag=f"recB{idx}", name="recB")
                        nc.gpsimd.partition_broadcast(recB[:, :], rec[0:1, :])
                        nc.vector.tensor_mul(
                            otb[hp][hh * DH:(hh + 1) * DH, qoff:qoff + QCH],
                            ot[0:DH, :],
                            recB[:, :],
                        )
                    # spread the first projection half across qc=1 segment
                    # boundaries: one q-tile after each of hp=0..3, so ScalarE
                    # drains its exp backlog during each insertion instead of
                    # idling through one 8us block
                    if qc == 1 and hp < 4:
                        proj_half(hp, hp + 1)
            proj_half(NQ // P // 2, NQ // P)

    nc.compile()
    return nc


def _get_prog() -> bass.Bass:
    global _PROG
    if _PROG is None:
        _PROG = _build_program()
    return _PROG


def kernel(x, Wq, Wk, Wv, Wp, bp):
    global LAST_RESULT
    import ml_dtypes
    BF = ml_dtypes.bfloat16
    x = np.asarray(x, dtype=np.float32)
    wqt = np.ascontiguousarray(np.asarray(Wq, np.float32).T.astype(BF))
    wkt = np.ascontiguousarray(np.asarray(Wk, np.float32).T.astype(BF))
    wvt = np.asarray(Wv, np.float32).T.astype(BF)
    wpt = np.ascontiguousarray(np.asarray(Wp, np.float32).T.astype(BF))
    bpv = np.ascontiguousarray(np.asarray(bp, np.float32).reshape(1, C))

    # interleave Wv columns to stride 65 (gap col is overwritten with ones
    # on-chip, so its weight values are irrelevant)
    wvi = np.zeros((C, VW), BF)
    for h in range(H):
        wvi[:, h * (DH + 1):h * (DH + 1) + DH] = wvt[:, h * DH:(h + 1) * DH]

    B, N, _ = x.shape
    in_maps = []
    for core in range(8):
        b, qh = core // 2, core % 2
        xt = x[b].T.astype(BF)
        xa = np.ascontiguousarray(xt[:, qh * NQ:(qh + 1) * NQ])
        xbm = np.ascontiguousarray(xt[:, (1 - qh) * NQ:(2 - qh) * NQ])
        in_maps.append({
            "xta": xa, "xtb": xbm,
            "wqt": wqt, "wkt": wkt, "wvi": wvi, "wpt": wpt, "bp": bpv,
        })

    res = run_bass_kernel_spmd(
        _get_prog(), in_maps, core_ids=list(range(8)),
        trace=bool(os.environ.get("BASS_TRACE")),
    )
    LAST_RESULT = res

    out = np.empty((B, N, C), np.float32)
    for core in range(8):
        b, qh = core // 2, core % 2
        out[b, qh * NQ:(qh + 1) * NQ, :] = res.results[core]["y"]
    return out
